# revision 24
# baseline (speedup 1.0000x reference)
"""RWKV WKV recurrence kernel for Trainium2 (8 NeuronCores).

Per core = one batch. Channels on partitions (16 groups of 128), time on
the free dim. v2 design (radix-2 time decimation):

- Inputs host-cast to fp16 (halves load DMA); output fp16, host-cast back.
- Loads phase-separate time: within each 128-step block, even steps land
  in partitions 0-63, odd in 64-127 ("(s p q) h -> (q p) s h").  After the
  PE transpose, each block's even/odd phases are 64-contiguous column
  runs, so phase views stay 2x-eligible on the DVE.
- Scale invariance: out = (A~_{t-1} + x_t)/(B~_{t-1} + y_t) with
  A~ = scan(x/eu), so the combine adds read the RAW transposed values
  straight from PSUM and all per-channel scalings (1/eu, lam/eu, lam)
  ride ACT copies.
- Radix-2: the DVE scan (2 cycles/col, measured invariant to operand
  placement/dtype) runs only over odd positions with coefficient lam^2 on
  z_j = lam*x_{2j}/eu + x_{2j+1}/eu; even positions are reconstructed
  with 2x-mode tensor_tensor adds. Halves scan columns: 140us -> 70us.
- Fused custom-DVE divide (bitwise-NOT seed + deg-2 minimax poly) writes
  its output through a natural-time-order scatter AP, so the output path
  (PE transposes + ACT copies + pair-wide 512B-row stores) is standard.
- PSUM residency trick: w = x_e/eu + x_o is formed EARLY (while the
  PSUM transposes are fresh), so the odd-phase combine later is
  SBUF-only and the next group's PE transposes overlap this group's
  tail.  Without this the per-group chains serialize (337us vs 278us).

Measured on HW: ~278us (baseline v1: ~298us), rel err 2.0e-3.
Engine balance: DVE ~221us busy (scans 73 + adds 103 + divide 42),
ACT ~205us, PE ~93us, DMA input-paced at ~63GB/s effective (512B rows).
"""

import os
import sys
from contextlib import ExitStack

import numpy as np

for _p in ("/opt/trn_rl_repo", "/root/.axon_site/_ro/trn_rl_repo"):
    if os.path.isdir(_p) and _p not in sys.path:
        sys.path.insert(0, _p)

import concourse.bacc as bacc
import concourse.mybir as mybir
import concourse.tile as tile
from concourse import dve_ops as _dve_ops
from concourse import dve_spec as _dve_spec
from concourse import masks
from concourse.bass_utils import run_bass_kernel_spmd
from concourse.dve_spec import Spec as _Spec, lower as _dve_lower
from concourse.dve_uop import AluOp as _AluOp, DveOpSpec as _DveOpSpec

F32 = mybir.dt.float32
F16 = mybir.dt.float16
AF = mybir.ActivationFunctionType
OP = mybir.AluOpType

_DIV_C0, _DIV_C1, _DIV_C2 = -0.7071067, -0.1665221, -0.013060556


def _div_mul_ref(in0, in1, c0, c1, c2):
    in0 = np.asarray(in0, np.float32)
    in1 = np.asarray(in1, np.float32)
    n = (~in0.view(np.int32)).view(np.float32)
    s = (in0 * n).astype(np.float32)
    q = (in1 * n).astype(np.float32)
    u = (s * np.float32(c2)).astype(np.float32)
    v = (np.float32(c1) + u).astype(np.float32)
    w = (s * v).astype(np.float32)
    p = (np.float32(c0) + w).astype(np.float32)
    return (q * p).astype(np.float32)


def _register_div_mul():
    name = "WKV_DIV_MUL_ANT"
    if name in _dve_ops._SUB_OPCODE_FOR_NAME:
        return next(o for o in _dve_ops.OPS if o.name == name)
    Src0, Src1 = _dve_spec.Src0, _dve_spec.Src1
    C0, C1, C2 = _dve_spec.C0, _dve_spec.C1, _dve_spec.C2
    _n = _dve_spec.Bin(_AluOp.BITWISE_NOT, Src0, Src0)
    _s = Src0 * _n
    body = (Src1 * _n) * (C0 + _s * (C1 + _s * C2))
    spec = _Spec(body=body, reference=_div_mul_ref)
    shas = {}
    for ver in ("v3", "v4"):
        try:
            uops = _dve_lower(spec, ver=ver)
        except Exception:
            continue
        shas[ver] = _DveOpSpec(name=name, opcode=0, uops=uops, rd1_en=True).sha(ver)
    op = _dve_ops.DveOp(name, spec, subdim=False, uops_sha=shas)
    row = _dve_ops._CUSTOM_DVE_ROW_BASE + len(_dve_ops.OPS)
    assert row < 0x20
    _dve_ops.OPS.append(op)
    _dve_ops._SUB_OPCODE_FOR_NAME[name] = row
    _dve_ops.CUSTOM_DVE_SPECS[name] = spec
    return op


WKV_DIV_MUL = _register_div_mul()

B, T, H = 8, 2048, 2048
N_CORES = 8


def build_nc(t=T, h=H):
    nc = bacc.Bacc("TRN2", target_bir_lowering=False, debug=False)

    key = nc.dram_tensor("key", [t, h], F16, kind="ExternalInput").ap()
    value = nc.dram_tensor("value", [t, h], F16, kind="ExternalInput").ap()
    td = nc.dram_tensor("time_decay", [h], F32, kind="ExternalInput").ap()
    tf = nc.dram_tensor("time_first", [h], F32, kind="ExternalInput").ap()
    out = nc.dram_tensor("out", [t, h], F16, kind="ExternalOutput").ap()

    G = h // 128  # channel groups
    S = t // 128  # time blocks per group
    NP = G // 2  # group pairs
    J = t // 2  # radix-2 pairs per group (1024)

    with tile.TileContext(nc) as tc, ExitStack() as ctx:
        const = ctx.enter_context(tc.tile_pool(name="const", bufs=1))
        ident = const.tile([128, 128], F16)
        masks.make_identity(nc, ident[:])

        # per-channel constants, [128, G] layout (partition = ch%128)
        tf_t = const.tile([128, G], F32)
        nc.sync.dma_start(tf_t[:], tf.rearrange("(g p) -> p g", p=128))
        td_t = const.tile([128, G], F32)
        nc.sync.dma_start(td_t[:], td.rearrange("(g p) -> p g", p=128))
        etd_t = const.tile([128, G], F32)
        nc.scalar.activation(etd_t[:], td_t[:], AF.Exp)
        lam_t = const.tile([128, G], F32)  # lam = exp(-exp(td))
        nc.scalar.activation(lam_t[:], etd_t[:], AF.Exp, scale=-1.0)
        ieu_t = const.tile([128, G], F32)  # 1/eu = exp(-tf)
        nc.scalar.activation(ieu_t[:], tf_t[:], AF.Exp, scale=-1.0)
        lamieu_t = const.tile([128, G], F32)  # lam/eu
        nc.vector.tensor_mul(lamieu_t[:], lam_t[:], ieu_t[:])
        lam2_t = const.tile([128, G], F16)  # lam^2 (fp16, for the scan)
        nc.vector.tensor_mul(lam2_t[:], lam_t[:], lam_t[:])

        kin = ctx.enter_context(tc.tile_pool(name="kin", bufs=3))
        tm = ctx.enter_context(tc.tile_pool(name="tm", bufs=3))
        mid = ctx.enter_context(tc.tile_pool(name="mid", bufs=2))
        op_pool = ctx.enter_context(tc.tile_pool(name="op", bufs=2))
        ost_pool = ctx.enter_context(tc.tile_pool(name="ost", bufs=2))
        psk = ctx.enter_context(tc.tile_pool(name="psk", bufs=1, space="PSUM"))
        psv = ctx.enter_context(tc.tile_pool(name="psv", bufs=1, space="PSUM"))
        pso = ctx.enter_context(tc.tile_pool(name="pso", bufs=2, space="PSUM"))

        for pg in range(NP):
            h2 = slice(pg * 256, (pg + 1) * 256)

            # ---- pair load, phase-separated partitions:
            # t = s*128 + p*2 + q  ->  partition q*64+p, free (s, ch)
            kc = kin.tile([128, 2 * t], F16, tag="kc")
            vc = kin.tile([128, 2 * t], F16, tag="vc")
            ekt = tm.tile([128, 2 * t], F16, tag="ekt")
            evt = tm.tile([128, 2 * t], F16, tag="evt")
            kc_s2 = kc[:].rearrange("p (s h) -> p s h", h=256)
            vc_s2 = vc[:].rearrange("p (s h) -> p s h", h=256)
            kv = key[:, h2].rearrange("(s p q) h -> q p s h", p=64, q=2)
            vv = value[:, h2].rearrange("(s p q) h -> q p s h", p=64, q=2)
            for q in range(2):
                ps_ = slice(q * 64, (q + 1) * 64)
                nc.sync.dma_start(kc_s2[ps_], kv[q])
                nc.scalar.dma_start(vc_s2[ps_], vv[q])
            if pg == 0:
                # first pair: exp/mul per group half so the g0 pipeline
                # starts as soon as its half of the pair is in
                ekt_s = ekt[:].rearrange("p (s h) -> p s h", h=256)
                evt_s = evt[:].rearrange("p (s h) -> p s h", h=256)
                kc_s = kc_s2
                vc_s = vc_s2
                for gh in range(2):
                    ghs = slice(gh * 128, (gh + 1) * 128)
                    nc.scalar.activation(ekt_s[:, :, ghs], kc_s[:, :, ghs], AF.Exp)
                    nc.vector.tensor_mul(
                        evt_s[:, :, ghs], ekt_s[:, :, ghs], vc_s[:, :, ghs]
                    )
            else:
                nc.scalar.activation(ekt[:], kc[:], AF.Exp)
                nc.vector.tensor_mul(evt[:], ekt[:], vc[:])

            ost = ost_pool.tile([128, 2 * t], F16, tag="ost")
            ost_s = ost[:].rearrange("p (s h) -> p s h", h=256)

            for g2 in range(2):
                g = 2 * pg + g2
                lam_g = lam_t[:, g : g + 1]
                ieu_g = ieu_t[:, g : g + 1]
                lamieu_g = lamieu_t[:, g : g + 1]
                lam2_g = lam2_t[:, g : g + 1]
                sfx = str(g2)

                # ---- transpose exp'd values into PSUM (raw operands) ----
                kT = psk.tile([128, t], F16, tag="kT")
                vT = psv.tile([128, t], F16, tag="vT")
                for s in range(S):
                    bs = slice(s * 256 + g2 * 128, s * 256 + (g2 + 1) * 128)
                    nc.tensor.transpose(
                        kT[:, s * 128 : (s + 1) * 128], ekt[:, bs], ident[:]
                    )
                    nc.tensor.transpose(
                        vT[:, s * 128 : (s + 1) * 128], evt[:, bs], ident[:]
                    )

                # phase views of PSUM raws: [128, (16 s, 64 p)]
                kT_p = kT[:].rearrange("c (s q p) -> c q s p", q=2, p=64)
                vT_p = vT[:].rearrange("c (s q p) -> c q s p", q=2, p=64)

                numden = mid.tile([128, 2 * t], F16, tag="numden")
                den = numden[:, 0 : t]
                num = numden[:, t : 2 * t]

                # Pass 1 (both sides): scaled copies, z/w, scan.  Pass 2:
                # the post-scan combines.  Splitting the passes hides each
                # side's cbt (ACT) latency behind the OTHER side's scan on
                # the in-order DVE queue.
                sides = ((kT, kT_p, den, "k"), (vT, vT_p, num, "v"))
                st = {}
                for P, P_p, res, tg in sides:
                    # u = P/eu, written PHASE-SEPARATED ([e|o] halves) so the
                    # z and den_o adds below read contiguous operands
                    u = mid.tile([128, t], F16, tag="u" + tg, name="u" + tg)
                    nc.scalar.mul(
                        u[:].rearrange("c (q s p) -> c q s p", q=2, p=64),
                        P[:].rearrange("c (s q p) -> c q s p", q=2, p=64),
                        ieu_g,
                    )
                    u_e, u_o = u[:, 0:J], u[:, J : 2 * J]
                    c1 = mid.tile([128, J], F16, tag="c1" + tg, name="c1" + tg)
                    nc.scalar.mul(
                        c1[:].rearrange("c (s p) -> c s p", p=64), P_p[:, 0], lamieu_g
                    )
                    # z = lam*x_e/eu + x_o/eu   (early)
                    z = mid.tile([128, J], F16, tag="z" + tg, name="z" + tg)
                    nc.vector.tensor_add(z[:], c1[:], u_o)
                    # w = x_e/eu + x_o (early PSUM read; den_o is then
                    # SBUF-only, shortening PSUM residency)
                    w = mid.tile([128, J], F16, tag="w" + tg, name="w" + tg)
                    nc.vector.tensor_add(
                        w[:].rearrange("c (s p) -> c s p", p=64),
                        u_e.rearrange("c (s p) -> c s p", p=64),
                        P_p[:, 1],
                    )
                    # scan over pairs, coeff lam^2 -> A~ at odd positions
                    AB = mid.tile([128, J + 1], F16, tag="AB" + tg, name="AB" + tg)
                    nc.gpsimd.memset(AB[:, 0:1], 0.0)
                    nc.vector.tensor_tensor_scan(
                        AB[:, 1 : J + 1],
                        lam2_g.broadcast_to((128, J)),
                        z[:],
                        0.0,
                        OP.mult,
                        OP.add,
                    )
                    st[tg] = (AB, w)

                for P, P_p, res, tg in sides:
                    AB, w = st[tg]
                    # even outputs: res_e = A~_{j-1} + x_e (last PSUM read)
                    nc.vector.tensor_add(
                        res[:, 0:J].rearrange("c (s p) -> c s p", p=64),
                        AB[:, 0:J].rearrange("c (s p) -> c s p", p=64),
                        P_p[:, 0],
                    )
                    # odd outputs: res_o = lam*A~_{j-1} + x_e/eu + x_o
                    cbt = mid.tile([128, J + 1], F16, tag="cbt" + tg, name="cbt" + tg)
                    nc.gpsimd.memset(cbt[:, 0:1], 0.0)
                    nc.scalar.mul(cbt[:, 1 : J + 1], AB[:, 1 : J + 1], lam_g)
                    nc.vector.tensor_add(
                        res[:, J : 2 * J], cbt[:, 0:J], w[:]
                    )

                # ---- fused divide; scatter output to natural time order:
                # stream order (q-major phases) -> col s*128 + p*2 + q
                outp = op_pool.tile([128, t], F16, tag="outp" + sfx)
                outp_ph = outp[:].rearrange("c (s p q) -> c q s p", p=64, q=2)
                for q in range(2):
                    nc.vector._custom_dve(
                        WKV_DIV_MUL,
                        out=outp_ph[:, q],
                        in0=den[:, q * J : (q + 1) * J],
                        in1=num[:, q * J : (q + 1) * J],
                        s0=_DIV_C0,
                        s1=_DIV_C1,
                        imm2=_DIV_C2,
                    )

                # ---- transpose back + copy into pair staging (fp16) ----
                oT = pso.tile([128, 2048], F16, tag="oT")
                for s in range(S):
                    nc.tensor.transpose(
                        oT[:, s * 128 : (s + 1) * 128],
                        outp[:, s * 128 : (s + 1) * 128],
                        ident[:],
                    )
                nc.scalar.copy(
                    ost_s[:, :, g2 * 128 : (g2 + 1) * 128],
                    oT[:].rearrange("p (s h) -> p s h", h=128),
                )

            nc.sync.dma_start(
                out[:, h2].rearrange("(s p) h -> p s h", p=128),
                ost[:].rearrange("p (s h) -> p s h", h=256),
            )

    nc.compile()
    return nc


_nc_cache = {}


def _get_nc():
    if "nc" not in _nc_cache:
        _nc_cache["nc"] = build_nc()
    return _nc_cache["nc"]


def kernel_with_results(key, value, time_decay, time_first, trace=False, tmpdir=None):
    nc = _get_nc()
    key = np.asarray(key, dtype=np.float32).astype(np.float16)
    value = np.asarray(value, dtype=np.float32).astype(np.float16)
    time_decay = np.ascontiguousarray(time_decay, dtype=np.float32)
    time_first = np.ascontiguousarray(time_first, dtype=np.float32)
    in_maps = [
        {
            "key": np.ascontiguousarray(key[i]),
            "value": np.ascontiguousarray(value[i]),
            "time_decay": time_decay,
            "time_first": time_first,
        }
        for i in range(N_CORES)
    ]
    res = run_bass_kernel_spmd(
        nc, in_maps, list(range(N_CORES)), trace=trace, tmpdir=tmpdir
    )
    out = np.stack(
        [res.results[i]["out"].astype(np.float32) for i in range(N_CORES)], axis=0
    )
    return out, res


def kernel(key, value, time_decay, time_first):
    out, _ = kernel_with_results(key, value, time_decay, time_first)
    return out


# revision 25
# speedup vs baseline: 1.0087x; 1.0087x over previous
"""RWKV WKV recurrence kernel for Trainium2 (8 NeuronCores).

Per core = one batch. Channels on partitions (16 groups of 128), time on
the free dim. v2 design (radix-2 time decimation):

- Inputs host-cast to fp16 (halves load DMA); output fp16, host-cast back.
- Loads phase-separate time: within each 128-step block, even steps land
  in partitions 0-63, odd in 64-127 ("(s p q) h -> (q p) s h").  After the
  PE transpose, each block's even/odd phases are 64-contiguous column
  runs, so phase views stay 2x-eligible on the DVE.
- Scale invariance: out = (A~_{t-1} + x_t)/(B~_{t-1} + y_t) with
  A~ = scan(x/eu), so the combine adds read the RAW transposed values
  straight from PSUM and all per-channel scalings (1/eu, lam/eu, lam)
  ride ACT copies.
- Radix-2: the DVE scan (2 cycles/col, measured invariant to operand
  placement/dtype) runs only over odd positions with coefficient lam^2 on
  z_j = lam*x_{2j}/eu + x_{2j+1}/eu; even positions are reconstructed
  with 2x-mode tensor_tensor adds. Halves scan columns: 140us -> 70us.
- Fused custom-DVE divide (bitwise-NOT seed + deg-2 minimax poly) writes
  its output through a natural-time-order scatter AP, so the output path
  (PE transposes + ACT copies + pair-wide 512B-row stores) is standard.
- PSUM residency trick: w = x_e/eu + x_o is formed EARLY (while the
  PSUM transposes are fresh), so the odd-phase combine later is
  SBUF-only and the next group's PE transposes overlap this group's
  tail.  Without this the per-group chains serialize (337us vs 278us).

Measured on HW: ~278us (baseline v1: ~298us), rel err 2.0e-3.
Engine balance: DVE ~221us busy (scans 73 + adds 103 + divide 42),
ACT ~205us, PE ~93us, DMA input-paced at ~63GB/s effective (512B rows).
"""

import os
import sys
from contextlib import ExitStack

import numpy as np

for _p in ("/opt/trn_rl_repo", "/root/.axon_site/_ro/trn_rl_repo"):
    if os.path.isdir(_p) and _p not in sys.path:
        sys.path.insert(0, _p)

import concourse.bacc as bacc
import concourse.mybir as mybir
import concourse.tile as tile
from concourse import dve_ops as _dve_ops
from concourse import dve_spec as _dve_spec
from concourse import masks
from concourse.bass_utils import run_bass_kernel_spmd
from concourse.dve_spec import Spec as _Spec, lower as _dve_lower
from concourse.dve_uop import AluOp as _AluOp, DveOpSpec as _DveOpSpec

F32 = mybir.dt.float32
F16 = mybir.dt.float16
AF = mybir.ActivationFunctionType
OP = mybir.AluOpType

_DIV_C0, _DIV_C1, _DIV_C2 = -0.7071067, -0.1665221, -0.013060556


def _div_mul_ref(in0, in1, c0, c1, c2):
    in0 = np.asarray(in0, np.float32)
    in1 = np.asarray(in1, np.float32)
    n = (~in0.view(np.int32)).view(np.float32)
    s = (in0 * n).astype(np.float32)
    q = (in1 * n).astype(np.float32)
    u = (s * np.float32(c2)).astype(np.float32)
    v = (np.float32(c1) + u).astype(np.float32)
    w = (s * v).astype(np.float32)
    p = (np.float32(c0) + w).astype(np.float32)
    return (q * p).astype(np.float32)


def _register_div_mul():
    name = "WKV_DIV_MUL_ANT"
    if name in _dve_ops._SUB_OPCODE_FOR_NAME:
        return next(o for o in _dve_ops.OPS if o.name == name)
    Src0, Src1 = _dve_spec.Src0, _dve_spec.Src1
    C0, C1, C2 = _dve_spec.C0, _dve_spec.C1, _dve_spec.C2
    _n = _dve_spec.Bin(_AluOp.BITWISE_NOT, Src0, Src0)
    _s = Src0 * _n
    body = (Src1 * _n) * (C0 + _s * (C1 + _s * C2))
    spec = _Spec(body=body, reference=_div_mul_ref)
    shas = {}
    for ver in ("v3", "v4"):
        try:
            uops = _dve_lower(spec, ver=ver)
        except Exception:
            continue
        shas[ver] = _DveOpSpec(name=name, opcode=0, uops=uops, rd1_en=True).sha(ver)
    op = _dve_ops.DveOp(name, spec, subdim=False, uops_sha=shas)
    row = _dve_ops._CUSTOM_DVE_ROW_BASE + len(_dve_ops.OPS)
    assert row < 0x20
    _dve_ops.OPS.append(op)
    _dve_ops._SUB_OPCODE_FOR_NAME[name] = row
    _dve_ops.CUSTOM_DVE_SPECS[name] = spec
    return op


WKV_DIV_MUL = _register_div_mul()

B, T, H = 8, 2048, 2048
N_CORES = 8


def build_nc(t=T, h=H):
    nc = bacc.Bacc("TRN2", target_bir_lowering=False, debug=False)

    key = nc.dram_tensor("key", [t, h], F16, kind="ExternalInput").ap()
    value = nc.dram_tensor("value", [t, h], F16, kind="ExternalInput").ap()
    td = nc.dram_tensor("time_decay", [h], F32, kind="ExternalInput").ap()
    tf = nc.dram_tensor("time_first", [h], F32, kind="ExternalInput").ap()
    out = nc.dram_tensor("out", [t, h], F16, kind="ExternalOutput").ap()

    G = h // 128  # channel groups
    S = t // 128  # time blocks per group
    NP = G // 2  # group pairs
    J = t // 2  # radix-2 pairs per group (1024)

    with tile.TileContext(nc) as tc, ExitStack() as ctx:
        const = ctx.enter_context(tc.tile_pool(name="const", bufs=1))
        ident = const.tile([128, 128], F16)
        masks.make_identity(nc, ident[:])

        # per-channel constants, [128, G] layout (partition = ch%128)
        tf_t = const.tile([128, G], F32)
        nc.sync.dma_start(tf_t[:], tf.rearrange("(g p) -> p g", p=128))
        td_t = const.tile([128, G], F32)
        nc.sync.dma_start(td_t[:], td.rearrange("(g p) -> p g", p=128))
        etd_t = const.tile([128, G], F32)
        nc.scalar.activation(etd_t[:], td_t[:], AF.Exp)
        lam_t = const.tile([128, G], F32)  # lam = exp(-exp(td))
        nc.scalar.activation(lam_t[:], etd_t[:], AF.Exp, scale=-1.0)
        ieu_t = const.tile([128, G], F32)  # 1/eu = exp(-tf)
        nc.scalar.activation(ieu_t[:], tf_t[:], AF.Exp, scale=-1.0)
        lamieu_t = const.tile([128, G], F32)  # lam/eu
        nc.vector.tensor_mul(lamieu_t[:], lam_t[:], ieu_t[:])
        lam2_t = const.tile([128, G], F16)  # lam^2 (fp16, for the scan)
        nc.vector.tensor_mul(lam2_t[:], lam_t[:], lam_t[:])

        kin = ctx.enter_context(tc.tile_pool(name="kin", bufs=3))
        tm = ctx.enter_context(tc.tile_pool(name="tm", bufs=3))
        mid = ctx.enter_context(tc.tile_pool(name="mid", bufs=2))
        op_pool = ctx.enter_context(tc.tile_pool(name="op", bufs=2))
        ost_pool = ctx.enter_context(tc.tile_pool(name="ost", bufs=2))
        psk = ctx.enter_context(tc.tile_pool(name="psk", bufs=1, space="PSUM"))
        psv = ctx.enter_context(tc.tile_pool(name="psv", bufs=1, space="PSUM"))
        pso = ctx.enter_context(tc.tile_pool(name="pso", bufs=2, space="PSUM"))

        for pg in range(NP):
            h2 = slice(pg * 256, (pg + 1) * 256)

            # ---- pair load, phase-separated partitions:
            # t = s*128 + p*2 + q  ->  partition q*64+p, free (s, ch)
            kc = kin.tile([128, 2 * t], F16, tag="kc")
            vc = kin.tile([128, 2 * t], F16, tag="vc")
            ekt = tm.tile([128, 2 * t], F16, tag="ekt")
            evt = tm.tile([128, 2 * t], F16, tag="evt")
            kc_s2 = kc[:].rearrange("p (s h) -> p s h", h=256)
            vc_s2 = vc[:].rearrange("p (s h) -> p s h", h=256)
            kv = key[:, h2].rearrange("(s p q) h -> q p s h", p=64, q=2)
            vv = value[:, h2].rearrange("(s p q) h -> q p s h", p=64, q=2)
            for q in range(2):
                ps_ = slice(q * 64, (q + 1) * 64)
                nc.sync.dma_start(kc_s2[ps_], kv[q])
                nc.scalar.dma_start(vc_s2[ps_], vv[q])
            if pg == 0:
                # first pair: exp/mul per group half so the g0 pipeline
                # starts as soon as its half of the pair is in
                ekt_s = ekt[:].rearrange("p (s h) -> p s h", h=256)
                evt_s = evt[:].rearrange("p (s h) -> p s h", h=256)
                kc_s = kc_s2
                vc_s = vc_s2
                for gh in range(2):
                    ghs = slice(gh * 128, (gh + 1) * 128)
                    nc.scalar.activation(ekt_s[:, :, ghs], kc_s[:, :, ghs], AF.Exp)
                    nc.vector.tensor_mul(
                        evt_s[:, :, ghs], ekt_s[:, :, ghs], vc_s[:, :, ghs]
                    )
            else:
                nc.scalar.activation(ekt[:], kc[:], AF.Exp)
                nc.vector.tensor_mul(evt[:], ekt[:], vc[:])

            ost = ost_pool.tile([128, 2 * t], F16, tag="ost")
            ost_s = ost[:].rearrange("p (s h) -> p s h", h=256)

            for g2 in range(2):
                g = 2 * pg + g2
                lam_g = lam_t[:, g : g + 1]
                ieu_g = ieu_t[:, g : g + 1]
                lamieu_g = lamieu_t[:, g : g + 1]
                lam2_g = lam2_t[:, g : g + 1]
                sfx = str(g2)

                # ---- transpose exp'd values into PSUM (raw operands) ----
                kT = psk.tile([128, t], F16, tag="kT")
                vT = psv.tile([128, t], F16, tag="vT")
                for s in range(S):
                    bs = slice(s * 256 + g2 * 128, s * 256 + (g2 + 1) * 128)
                    nc.tensor.transpose(
                        kT[:, s * 128 : (s + 1) * 128], ekt[:, bs], ident[:]
                    )
                    nc.tensor.transpose(
                        vT[:, s * 128 : (s + 1) * 128], evt[:, bs], ident[:]
                    )

                # phase views of PSUM raws: [128, (16 s, 64 p)]
                kT_p = kT[:].rearrange("c (s q p) -> c q s p", q=2, p=64)
                vT_p = vT[:].rearrange("c (s q p) -> c q s p", q=2, p=64)

                numden = mid.tile([128, 2 * t], F16, tag="numden")
                den = numden[:, 0 : t]
                num = numden[:, t : 2 * t]

                for P, P_p, res in ((kT, kT_p, den), (vT, vT_p, num)):
                    tg = "k" if res is den else "v"
                    # scaled copies (ACT, early PSUM reads):
                    #   u = P/eu (full), c1 = lam*P_e/eu
                    # u = P/eu, written PHASE-SEPARATED ([e|o] halves) so the
                    # z and den_o adds below read contiguous operands
                    u = mid.tile([128, t], F16, tag="u" + tg)
                    nc.scalar.mul(
                        u[:].rearrange("c (q s p) -> c q s p", q=2, p=64),
                        P[:].rearrange("c (s q p) -> c q s p", q=2, p=64),
                        ieu_g,
                    )
                    u_e, u_o = u[:, 0:J], u[:, J : 2 * J]
                    c1 = mid.tile([128, J], F16, tag="c1" + tg)
                    nc.scalar.mul(
                        c1[:].rearrange("c (s p) -> c s p", p=64), P_p[:, 0], lamieu_g
                    )
                    # z = lam*x_e/eu + x_o/eu   (early PSUM read)
                    z = mid.tile([128, J], F16, tag="z" + tg)
                    nc.vector.tensor_add(z[:], c1[:], u_o)
                    # w = x_e/eu + x_o (early PSUM read; den_o = cbt_sh + w
                    # later is then SBUF-only, shortening PSUM residency)
                    w = mid.tile([128, J], F16, tag="w" + tg)
                    nc.vector.tensor_add(
                        w[:].rearrange("c (s p) -> c s p", p=64),
                        u_e.rearrange("c (s p) -> c s p", p=64),
                        P_p[:, 1],
                    )
                    # scan over pairs, coeff lam^2 -> A~ at odd positions
                    AB = mid.tile([128, J + 1], F16, tag="AB" + tg)
                    nc.gpsimd.memset(AB[:, 0:1], 0.0)
                    nc.vector.tensor_tensor_scan(
                        AB[:, 1 : J + 1],
                        lam2_g.broadcast_to((128, J)),
                        z[:],
                        0.0,
                        OP.mult,
                        OP.add,
                    )
                    # even outputs: res_e = A~_{j-1} + x_e (last PSUM read)
                    nc.vector.tensor_add(
                        res[:, 0:J].rearrange("c (s p) -> c s p", p=64),
                        AB[:, 0:J].rearrange("c (s p) -> c s p", p=64),
                        P_p[:, 0],
                    )
                    # odd outputs: res_o = lam*A~_{j-1} + x_e/eu + x_o
                    cbt = mid.tile([128, J + 1], F16, tag="cbt" + tg)
                    nc.gpsimd.memset(cbt[:, 0:1], 0.0)
                    nc.scalar.mul(cbt[:, 1 : J + 1], AB[:, 1 : J + 1], lam_g)
                    nc.vector.tensor_add(
                        res[:, J : 2 * J], cbt[:, 0:J], w[:]
                    )

                # ---- fused divide; scatter output to natural time order:
                # stream order (q-major phases) -> col s*128 + p*2 + q
                outp = op_pool.tile([128, t], F16, tag="outp" + sfx)
                outp_ph = outp[:].rearrange("c (s p q) -> c q s p", p=64, q=2)
                for q in range(2):
                    nc.vector._custom_dve(
                        WKV_DIV_MUL,
                        out=outp_ph[:, q],
                        in0=den[:, q * J : (q + 1) * J],
                        in1=num[:, q * J : (q + 1) * J],
                        s0=_DIV_C0,
                        s1=_DIV_C1,
                        imm2=_DIV_C2,
                    )

                # ---- transpose back + copy into pair staging (fp16) ----
                oT = pso.tile([128, 2048], F16, tag="oT")
                for s in range(S):
                    nc.tensor.transpose(
                        oT[:, s * 128 : (s + 1) * 128],
                        outp[:, s * 128 : (s + 1) * 128],
                        ident[:],
                    )
                nc.scalar.copy(
                    ost_s[:, :, g2 * 128 : (g2 + 1) * 128],
                    oT[:].rearrange("p (s h) -> p s h", h=128),
                )

            nc.sync.dma_start(
                out[:, h2].rearrange("(s p) h -> p s h", p=128),
                ost[:].rearrange("p (s h) -> p s h", h=256),
            )

    nc.compile()
    return nc


_nc_cache = {}


def _get_nc():
    if "nc" not in _nc_cache:
        _nc_cache["nc"] = build_nc()
    return _nc_cache["nc"]


def kernel_with_results(key, value, time_decay, time_first, trace=False, tmpdir=None):
    nc = _get_nc()
    key = np.asarray(key, dtype=np.float32).astype(np.float16)
    value = np.asarray(value, dtype=np.float32).astype(np.float16)
    time_decay = np.ascontiguousarray(time_decay, dtype=np.float32)
    time_first = np.ascontiguousarray(time_first, dtype=np.float32)
    in_maps = [
        {
            "key": np.ascontiguousarray(key[i]),
            "value": np.ascontiguousarray(value[i]),
            "time_decay": time_decay,
            "time_first": time_first,
        }
        for i in range(N_CORES)
    ]
    res = run_bass_kernel_spmd(
        nc, in_maps, list(range(N_CORES)), trace=trace, tmpdir=tmpdir
    )
    out = np.stack(
        [res.results[i]["out"].astype(np.float32) for i in range(N_CORES)], axis=0
    )
    return out, res


def kernel(key, value, time_decay, time_first):
    out, _ = kernel_with_results(key, value, time_decay, time_first)
    return out


# revision 26
# speedup vs baseline: 1.0248x; 1.0160x over previous
"""RWKV WKV recurrence kernel for Trainium2 (8 NeuronCores).

Per core = one batch. Channels on partitions (16 groups of 128), time on
the free dim. v2 design (radix-2 time decimation):

- Inputs host-cast to fp16 (halves load DMA); output fp16, host-cast back.
- Loads phase-separate time: within each 128-step block, even steps land
  in partitions 0-63, odd in 64-127 ("(s p q) h -> (q p) s h").  After the
  PE transpose, each block's even/odd phases are 64-contiguous column
  runs, so phase views stay 2x-eligible on the DVE.
- Scale invariance: out = (A~_{t-1} + x_t)/(B~_{t-1} + y_t) with
  A~ = scan(x/eu), so the combine adds read the RAW transposed values
  straight from PSUM and all per-channel scalings (1/eu, lam/eu, lam)
  ride ACT copies.
- Radix-2: the DVE scan (2 cycles/col, measured invariant to operand
  placement/dtype) runs only over odd positions with coefficient lam^2 on
  z_j = lam*x_{2j}/eu + x_{2j+1}/eu; even positions are reconstructed
  with 2x-mode tensor_tensor adds. Halves scan columns: 140us -> 70us.
- Fused custom-DVE divide (bitwise-NOT seed + deg-2 minimax poly) writes
  its output through a natural-time-order scatter AP, so the output path
  (PE transposes + ACT copies + pair-wide 512B-row stores) is standard.
- PSUM residency trick: w = x_e/eu + x_o is formed EARLY (while the
  PSUM transposes are fresh), so the odd-phase combine later is
  SBUF-only and the next group's PE transposes overlap this group's
  tail.  Without this the per-group chains serialize (337us vs 278us).

Measured on HW: ~278us (baseline v1: ~298us), rel err 2.0e-3.
Engine balance: DVE ~221us busy (scans 73 + adds 103 + divide 42),
ACT ~205us, PE ~93us, DMA input-paced at ~63GB/s effective (512B rows).
"""

import os
import sys
from contextlib import ExitStack

import numpy as np

for _p in ("/opt/trn_rl_repo", "/root/.axon_site/_ro/trn_rl_repo"):
    if os.path.isdir(_p) and _p not in sys.path:
        sys.path.insert(0, _p)

import concourse.bacc as bacc
import concourse.mybir as mybir
import concourse.tile as tile
from concourse import dve_ops as _dve_ops
from concourse import dve_spec as _dve_spec
from concourse import masks
from concourse.bass_utils import run_bass_kernel_spmd
from concourse.dve_spec import Spec as _Spec, lower as _dve_lower
from concourse.dve_uop import AluOp as _AluOp, DveOpSpec as _DveOpSpec

F32 = mybir.dt.float32
F16 = mybir.dt.float16
AF = mybir.ActivationFunctionType
OP = mybir.AluOpType

_DIV_C0, _DIV_C1, _DIV_C2 = -0.7071067, -0.1665221, -0.013060556


def _div_mul_ref(in0, in1, c0, c1, c2):
    in0 = np.asarray(in0, np.float32)
    in1 = np.asarray(in1, np.float32)
    n = (~in0.view(np.int32)).view(np.float32)
    s = (in0 * n).astype(np.float32)
    q = (in1 * n).astype(np.float32)
    u = (s * np.float32(c2)).astype(np.float32)
    v = (np.float32(c1) + u).astype(np.float32)
    w = (s * v).astype(np.float32)
    p = (np.float32(c0) + w).astype(np.float32)
    return (q * p).astype(np.float32)


def _register_div_mul():
    name = "WKV_DIV_MUL_ANT"
    if name in _dve_ops._SUB_OPCODE_FOR_NAME:
        return next(o for o in _dve_ops.OPS if o.name == name)
    Src0, Src1 = _dve_spec.Src0, _dve_spec.Src1
    C0, C1, C2 = _dve_spec.C0, _dve_spec.C1, _dve_spec.C2
    _n = _dve_spec.Bin(_AluOp.BITWISE_NOT, Src0, Src0)
    _s = Src0 * _n
    body = (Src1 * _n) * (C0 + _s * (C1 + _s * C2))
    spec = _Spec(body=body, reference=_div_mul_ref)
    shas = {}
    for ver in ("v3", "v4"):
        try:
            uops = _dve_lower(spec, ver=ver)
        except Exception:
            continue
        shas[ver] = _DveOpSpec(name=name, opcode=0, uops=uops, rd1_en=True).sha(ver)
    op = _dve_ops.DveOp(name, spec, subdim=False, uops_sha=shas)
    row = _dve_ops._CUSTOM_DVE_ROW_BASE + len(_dve_ops.OPS)
    assert row < 0x20
    _dve_ops.OPS.append(op)
    _dve_ops._SUB_OPCODE_FOR_NAME[name] = row
    _dve_ops.CUSTOM_DVE_SPECS[name] = spec
    return op


WKV_DIV_MUL = _register_div_mul()

B, T, H = 8, 2048, 2048
N_CORES = 8


def build_nc(t=T, h=H):
    nc = bacc.Bacc("TRN2", target_bir_lowering=False, debug=False)

    key = nc.dram_tensor("key", [t, h], F16, kind="ExternalInput").ap()
    value = nc.dram_tensor("value", [t, h], F16, kind="ExternalInput").ap()
    td = nc.dram_tensor("time_decay", [h], F32, kind="ExternalInput").ap()
    tf = nc.dram_tensor("time_first", [h], F32, kind="ExternalInput").ap()
    out = nc.dram_tensor("out", [t, h], F16, kind="ExternalOutput").ap()

    G = h // 128  # channel groups
    S = t // 128  # time blocks per group
    NP = G // 2  # group pairs
    J = t // 2  # radix-2 pairs per group (1024)

    with tile.TileContext(nc) as tc, ExitStack() as ctx:
        const = ctx.enter_context(tc.tile_pool(name="const", bufs=1))
        ident = const.tile([128, 128], F16)
        masks.make_identity(nc, ident[:])

        # per-channel constants, [128, G] layout (partition = ch%128)
        tf_t = const.tile([128, G], F32)
        nc.sync.dma_start(tf_t[:], tf.rearrange("(g p) -> p g", p=128))
        td_t = const.tile([128, G], F32)
        nc.sync.dma_start(td_t[:], td.rearrange("(g p) -> p g", p=128))
        etd_t = const.tile([128, G], F32)
        nc.scalar.activation(etd_t[:], td_t[:], AF.Exp)
        lam_t = const.tile([128, G], F32)  # lam = exp(-exp(td))
        nc.scalar.activation(lam_t[:], etd_t[:], AF.Exp, scale=-1.0)
        ieu_t = const.tile([128, G], F32)  # 1/eu = exp(-tf)
        nc.scalar.activation(ieu_t[:], tf_t[:], AF.Exp, scale=-1.0)
        lamieu_t = const.tile([128, G], F32)  # lam/eu
        nc.vector.tensor_mul(lamieu_t[:], lam_t[:], ieu_t[:])
        lam2_t = const.tile([128, G], F16)  # lam^2 (fp16, for the scan)
        nc.vector.tensor_mul(lam2_t[:], lam_t[:], lam_t[:])

        kin = ctx.enter_context(tc.tile_pool(name="kin", bufs=3))
        tm = ctx.enter_context(tc.tile_pool(name="tm", bufs=3))
        mid = ctx.enter_context(tc.tile_pool(name="mid", bufs=2))
        op_pool = ctx.enter_context(tc.tile_pool(name="op", bufs=2))
        ost_pool = ctx.enter_context(tc.tile_pool(name="ost", bufs=2))
        psk = ctx.enter_context(tc.tile_pool(name="psk", bufs=1, space="PSUM"))
        psv = ctx.enter_context(tc.tile_pool(name="psv", bufs=1, space="PSUM"))
        pso = ctx.enter_context(tc.tile_pool(name="pso", bufs=2, space="PSUM"))

        for pg in range(NP):
            h2 = slice(pg * 256, (pg + 1) * 256)

            # ---- pair load, phase-separated partitions:
            # t = s*128 + p*2 + q  ->  partition q*64+p, free (s, ch)
            kc = kin.tile([128, 2 * t], F16, tag="kc")
            vc = kin.tile([128, 2 * t], F16, tag="vc")
            ekt = tm.tile([128, 2 * t], F16, tag="ekt")
            evt = tm.tile([128, 2 * t], F16, tag="evt")
            kc_s2 = kc[:].rearrange("p (s h) -> p s h", h=256)
            vc_s2 = vc[:].rearrange("p (s h) -> p s h", h=256)
            if pg == 0:
                # natural-order channel-half loads (512B rows): group 0's
                # pipeline starts after HALF the pair data; phase views of
                # this pair are stride-2 (1x adds) -- warmup trade.
                for gh in range(2):
                    hh = slice(gh * 128, (gh + 1) * 128)
                    ghs = slice(gh * 128, (gh + 1) * 128)
                    nc.sync.dma_start(
                        kc_s2[:, :, ghs],
                        key[:, hh].rearrange("(s p) h -> p s h", p=128),
                    )
                    nc.scalar.dma_start(
                        vc_s2[:, :, ghs],
                        value[:, hh].rearrange("(s p) h -> p s h", p=128),
                    )
            else:
                kv = key[:, h2].rearrange("(s p q) h -> q p s h", p=64, q=2)
                vv = value[:, h2].rearrange("(s p q) h -> q p s h", p=64, q=2)
                for q in range(2):
                    ps_ = slice(q * 64, (q + 1) * 64)
                    nc.sync.dma_start(kc_s2[ps_], kv[q])
                    nc.scalar.dma_start(vc_s2[ps_], vv[q])
            if pg == 0:
                # first pair: exp/mul per group half so the g0 pipeline
                # starts as soon as its half of the pair is in
                ekt_s = ekt[:].rearrange("p (s h) -> p s h", h=256)
                evt_s = evt[:].rearrange("p (s h) -> p s h", h=256)
                kc_s = kc_s2
                vc_s = vc_s2
                for gh in range(2):
                    ghs = slice(gh * 128, (gh + 1) * 128)
                    nc.scalar.activation(ekt_s[:, :, ghs], kc_s[:, :, ghs], AF.Exp)
                    nc.vector.tensor_mul(
                        evt_s[:, :, ghs], ekt_s[:, :, ghs], vc_s[:, :, ghs]
                    )
            else:
                nc.scalar.activation(ekt[:], kc[:], AF.Exp)
                nc.vector.tensor_mul(evt[:], ekt[:], vc[:])

            ost = ost_pool.tile([128, 2 * t], F16, tag="ost")
            ost_s = ost[:].rearrange("p (s h) -> p s h", h=256)

            for g2 in range(2):
                g = 2 * pg + g2
                lam_g = lam_t[:, g : g + 1]
                ieu_g = ieu_t[:, g : g + 1]
                lamieu_g = lamieu_t[:, g : g + 1]
                lam2_g = lam2_t[:, g : g + 1]
                sfx = str(g2)

                # ---- transpose exp'd values into PSUM (raw operands) ----
                kT = psk.tile([128, t], F16, tag="kT")
                vT = psv.tile([128, t], F16, tag="vT")
                for s in range(S):
                    bs = slice(s * 256 + g2 * 128, s * 256 + (g2 + 1) * 128)
                    nc.tensor.transpose(
                        kT[:, s * 128 : (s + 1) * 128], ekt[:, bs], ident[:]
                    )
                    nc.tensor.transpose(
                        vT[:, s * 128 : (s + 1) * 128], evt[:, bs], ident[:]
                    )

                # phase views of PSUM raws: [128, (16 s, 64 p)] runs for
                # phase-separated pairs; stride-2 for the natural pair 0
                _ord = "c (s p q) -> c q s p" if pg == 0 else "c (s q p) -> c q s p"
                kT_p = kT[:].rearrange(_ord, q=2, p=64)
                vT_p = vT[:].rearrange(_ord, q=2, p=64)

                numden = mid.tile([128, 2 * t], F16, tag="numden")
                den = numden[:, 0 : t]
                num = numden[:, t : 2 * t]

                for P, P_p, res in ((kT, kT_p, den), (vT, vT_p, num)):
                    tg = "k" if res is den else "v"
                    # scaled copies (ACT, early PSUM reads):
                    #   u = P/eu (full), c1 = lam*P_e/eu
                    # u = P/eu, written PHASE-SEPARATED ([e|o] halves) so the
                    # z and den_o adds below read contiguous operands
                    u = mid.tile([128, t], F16, tag="u" + tg)
                    nc.scalar.mul(
                        u[:].rearrange("c (q s p) -> c q s p", q=2, p=64),
                        P[:].rearrange(_ord, q=2, p=64),
                        ieu_g,
                    )
                    u_e, u_o = u[:, 0:J], u[:, J : 2 * J]
                    c1 = mid.tile([128, J], F16, tag="c1" + tg)
                    nc.scalar.mul(
                        c1[:].rearrange("c (s p) -> c s p", p=64), P_p[:, 0], lamieu_g
                    )
                    # z = lam*x_e/eu + x_o/eu   (early PSUM read)
                    z = mid.tile([128, J], F16, tag="z" + tg)
                    nc.vector.tensor_add(z[:], c1[:], u_o)
                    # w = x_e/eu + x_o (early PSUM read; den_o = cbt_sh + w
                    # later is then SBUF-only, shortening PSUM residency)
                    w = mid.tile([128, J], F16, tag="w" + tg)
                    nc.vector.tensor_add(
                        w[:].rearrange("c (s p) -> c s p", p=64),
                        u_e.rearrange("c (s p) -> c s p", p=64),
                        P_p[:, 1],
                    )
                    # scan over pairs, coeff lam^2 -> A~ at odd positions
                    AB = mid.tile([128, J + 1], F16, tag="AB" + tg)
                    nc.gpsimd.memset(AB[:, 0:1], 0.0)
                    nc.vector.tensor_tensor_scan(
                        AB[:, 1 : J + 1],
                        lam2_g.broadcast_to((128, J)),
                        z[:],
                        0.0,
                        OP.mult,
                        OP.add,
                    )
                    # even outputs: res_e = A~_{j-1} + x_e (last PSUM read)
                    nc.vector.tensor_add(
                        res[:, 0:J].rearrange("c (s p) -> c s p", p=64),
                        AB[:, 0:J].rearrange("c (s p) -> c s p", p=64),
                        P_p[:, 0],
                    )
                    # odd outputs: res_o = lam*A~_{j-1} + x_e/eu + x_o
                    cbt = mid.tile([128, J + 1], F16, tag="cbt" + tg)
                    nc.gpsimd.memset(cbt[:, 0:1], 0.0)
                    nc.scalar.mul(cbt[:, 1 : J + 1], AB[:, 1 : J + 1], lam_g)
                    nc.vector.tensor_add(
                        res[:, J : 2 * J], cbt[:, 0:J], w[:]
                    )

                # ---- fused divide; scatter output to natural time order:
                # stream order (q-major phases) -> col s*128 + p*2 + q
                outp = op_pool.tile([128, t], F16, tag="outp" + sfx)
                _oord = "c (s p q) -> c q s p"  # natural scatter (pair 0: q is
                # the innermost time bit anyway; phase-sep pairs: partition
                # order was (q p), so the SAME mapping applies)
                outp_ph = outp[:].rearrange(_oord, p=64, q=2)
                for q in range(2):
                    nc.vector._custom_dve(
                        WKV_DIV_MUL,
                        out=outp_ph[:, q],
                        in0=den[:, q * J : (q + 1) * J],
                        in1=num[:, q * J : (q + 1) * J],
                        s0=_DIV_C0,
                        s1=_DIV_C1,
                        imm2=_DIV_C2,
                    )

                # ---- transpose back + copy into pair staging (fp16) ----
                oT = pso.tile([128, 2048], F16, tag="oT")
                for s in range(S):
                    nc.tensor.transpose(
                        oT[:, s * 128 : (s + 1) * 128],
                        outp[:, s * 128 : (s + 1) * 128],
                        ident[:],
                    )
                nc.scalar.copy(
                    ost_s[:, :, g2 * 128 : (g2 + 1) * 128],
                    oT[:].rearrange("p (s h) -> p s h", h=128),
                )

            nc.sync.dma_start(
                out[:, h2].rearrange("(s p) h -> p s h", p=128),
                ost[:].rearrange("p (s h) -> p s h", h=256),
            )

    nc.compile()
    return nc


_nc_cache = {}


def _get_nc():
    if "nc" not in _nc_cache:
        _nc_cache["nc"] = build_nc()
    return _nc_cache["nc"]


def kernel_with_results(key, value, time_decay, time_first, trace=False, tmpdir=None):
    nc = _get_nc()
    key = np.asarray(key, dtype=np.float32).astype(np.float16)
    value = np.asarray(value, dtype=np.float32).astype(np.float16)
    time_decay = np.ascontiguousarray(time_decay, dtype=np.float32)
    time_first = np.ascontiguousarray(time_first, dtype=np.float32)
    in_maps = [
        {
            "key": np.ascontiguousarray(key[i]),
            "value": np.ascontiguousarray(value[i]),
            "time_decay": time_decay,
            "time_first": time_first,
        }
        for i in range(N_CORES)
    ]
    res = run_bass_kernel_spmd(
        nc, in_maps, list(range(N_CORES)), trace=trace, tmpdir=tmpdir
    )
    out = np.stack(
        [res.results[i]["out"].astype(np.float32) for i in range(N_CORES)], axis=0
    )
    return out, res


def kernel(key, value, time_decay, time_first):
    out, _ = kernel_with_results(key, value, time_decay, time_first)
    return out
